# revision 24
# baseline (speedup 1.0000x reference)
"""Trainium2 Bass kernel for nn_NegF1: distributed -F1 loss over 16.7M elements.

Data-parallel over 8 NeuronCores; each core streams its 2,097,152-element
slice of probs (f32) / lbls (int32) from HBM (probs on the sync HWDGE ring,
lbls on the scalar ring). Memory-bound, so the structure guarantees the
input stream can never be throttled by compute:

  * Every input tile gets its OWN SBUF slot (no slot recycling; the full
    input fits in SBUF: 2 * 64KB per partition) and ALL input DMA triggers
    are issued up-front, before any compute instruction, so the DMA queues
    are full from the first microsecond and no consumer-side slot-release
    feedback ever gates a DMA.
  * Compute engines just chase the stream; each is far enough under the
    DMA rate to keep up even at its clock-gated slow p-state.

Key algebraic trick: with g = [p > .5], the F1 terms are
  FP = Y - TP,  FN = Npos - (Sx + C) + TP
so Sx = sum(l*p) and C = sum(l*g) are only needed as their SUM, which is
one diag plane z = g + p instead of two.

Work split per [128, F] tile (vs the 5.5us/tile DMA stream rate):
  ACT:  lb = bf16(l), fused accum -> Npos = sum(l)
  DVE:  stt z: (p > .5) + p -> comb[:, :F]          (plane only)
        stt y: (p > .5) * p -> comb[:, F:], accum -> Y
  PE:   diag trick: lhsT = lb chunk, rhs = [z_c | y_c] [128, 2*128] into
        2 alternating PSUM banks; diag block 0 -> Sx + C, block 1 -> TP.
A short burst of zero matmuls warms the PE clock during the first DMAs;
tiles taper at the end to shorten the drain.

Host combine (float64):
  FP = Y - TP;  FN = Npos - SxC + TP
  f1 from TP/FP/FN with eps=1e-5;  return -f1 as float32 scalar.
"""

from contextlib import ExitStack

import numpy as np

N_TOTAL = 16777216
N_CORES = 8
M_PER_CORE = N_TOTAL // N_CORES   # 2097152
P = 128                           # SBUF partitions
EPS = 1e-05
CH = 128                          # diag chunk columns

_CACHE = {}


def build_nc(M=M_PER_CORE, F=2048, comb_bufs=5, warmup_mms=12, debug=False):
    import concourse.bacc as bacc
    import concourse.mybir as mybir
    import concourse.tile as tile

    cols = M // P                 # 16384
    end_taper = [F // 2, F // 4, F // 8, F // 8]
    body = (cols - sum(end_taper)) // F
    tiles = [F] * body + end_taper
    assert sum(tiles) == cols and all(Ft % CH == 0 for Ft in tiles)
    T = len(tiles)

    f32 = mybir.dt.float32
    i32 = mybir.dt.int32
    bf16 = mybir.dt.bfloat16
    Alu = mybir.AluOpType
    Act = mybir.ActivationFunctionType

    nc = bacc.Bacc("TRN2", target_bir_lowering=False, debug=debug,
                   num_devices=N_CORES)

    probs = nc.dram_tensor("probs", [M], f32, kind="ExternalInput")
    lbls = nc.dram_tensor("lbls", [M], i32, kind="ExternalInput")
    out_diag = nc.dram_tensor("out_diag", [P, 2 * 2 * CH], bf16,
                              kind="ExternalOutput")
    out_accN = nc.dram_tensor("out_accN", [P, T], f32, kind="ExternalOutput")
    out_accY = nc.dram_tensor("out_accY", [P, T + 4], f32,
                              kind="ExternalOutput")

    # per-tile DRAM views: tile t is one contiguous block of P*Ft elements
    def tile_view(ap_flat, start_el, Ft):
        return ap_flat[start_el:start_el + P * Ft].rearrange(
            "(p f) -> p f", p=P, f=Ft)

    p1 = probs.ap()
    l1 = lbls.ap()

    with tile.TileContext(nc) as tc, ExitStack() as ctx:
        pin = ctx.enter_context(tc.tile_pool(name="pin", bufs=1))
        lin = ctx.enter_context(tc.tile_pool(name="lin", bufs=1))
        lbpool = ctx.enter_context(tc.tile_pool(name="lbpool", bufs=4))
        cpool = ctx.enter_context(tc.tile_pool(name="cpool", bufs=comb_bufs))
        accp = ctx.enter_context(tc.tile_pool(name="accp", bufs=1))
        psump = ctx.enter_context(tc.tile_pool(name="psump", bufs=1,
                                               space="PSUM"))

        accN = accp.tile([P, T], f32)        # Npos partials (ACT)
        # accY cols 0:T = Y partials; cols T:T+2 = SxC, T+2:T+4 = TP
        # partials of the DVE-tail tiles (all DVE-written)
        accY = accp.tile([P, T + 4], f32)

        # Phase 1: issue EVERY input DMA up-front, all on the SYNC ring.
        # Each tile has its own statically-assigned slot, so nothing ever
        # waits on a slot release and the stream can never be gated by
        # compute. The HWDGE ring-depth limit only blocks the sync
        # sequencer, which has nothing else to do -- the scalar sequencer
        # runs only ACT ops so they dispatch the moment their data lands.
        offs = []
        off = 0
        for Ft in tiles:
            offs.append(off)
            off += Ft
        pts = [pin.tile([P, Ft], f32, tag=f"pt{t}", name=f"pt{t}")
               for t, Ft in enumerate(tiles)]
        lts = [lin.tile([P, Ft], i32, tag=f"lt{t}", name=f"lt{t}")
               for t, Ft in enumerate(tiles)]

        def dma_p(t):
            nc.sync.dma_start(out=pts[t][:, :tiles[t]],
                              in_=tile_view(p1, P * offs[t], tiles[t]))

        def dma_l(t):
            nc.sync.dma_start(out=lts[t][:, :tiles[t]],
                              in_=tile_view(l1, P * offs[t], tiles[t]))

        # Stream order: a 4-tile probs head start (DVE depends only on
        # probs and can start immediately), then 1:1 interleave so the
        # ACT/PE chain warms up mid-stream and chases at a relaxed pace,
        # with no cold-start backlog left when the stream ends.
        HEAD = 4
        for t in range(min(HEAD, T)):
            dma_p(t)
        li = 0
        for t in range(HEAD, T):
            dma_l(li)
            li += 1
            dma_p(t)
        while li < T:
            dma_l(li)
            li += 1

        # two alternating diag accumulators (even / odd chunks) so
        # back-to-back accumulating matmuls don't chain on one PSUM bank
        ps_diag0 = psump.tile([P, 2 * CH], f32)
        ps_diag1 = psump.tile([P, 2 * CH], f32)

        # Warm the PE HAM clock-gate while the first input DMAs stream.
        if warmup_mms:
            wu = accp.tile([P, 2 * CH], bf16)
            nc.vector.memset(wu[:], 0.0)
            ps_wu = psump.tile([P, 2 * CH], f32)
            for i in range(warmup_mms):
                nc.tensor.matmul(ps_wu[:, :], wu[:, :CH], wu[:],
                                 start=(i == 0), stop=(i == warmup_mms - 1))

        # The last DVE_TAIL tiles keep their l-weighted sums on DVE, so
        # the PE diag stops (and PSUM drains) before the stream even ends.
        DVE_TAIL = 2
        nctot = sum(Ft for Ft in tiles[:-DVE_TAIL]) // CH
        bank = [i % 2 for i in range(nctot)]
        b0_stop = max(i for i, b in enumerate(bank) if b == 0)
        b1_stop = max(i for i, b in enumerate(bank) if b == 1)
        ci = 0
        diag_sb = accp.tile([P, 2 * 2 * CH], bf16)
        junk = accp.tile([P, F], bf16)

        # Phase 2: compute, chasing the stream.
        for t, Ft in enumerate(tiles):
            NCt = Ft // CH
            pt, lt = pts[t], lts[t]

            # ACT: lb = bf16(l) with fused accum -> Npos
            lb = lbpool.tile([P, F], bf16, tag="lb")
            nc.scalar.activation(lb[:, :Ft], lt[:, :Ft], Act.Copy,
                                 accum_out=accN[:, t:t + 1])

            # DVE: z = g + p plane; y = g * p plane with fused accum -> Y
            comb = cpool.tile([P, 2 * F], bf16, tag="comb")
            nc.vector.scalar_tensor_tensor(
                out=comb[:, :Ft], in0=pt[:, :Ft], scalar=0.5,
                in1=pt[:, :Ft], op0=Alu.is_gt, op1=Alu.add)
            nc.vector.scalar_tensor_tensor(
                out=comb[:, F:F + Ft], in0=pt[:, :Ft], scalar=0.5,
                in1=pt[:, :Ft], op0=Alu.is_gt, op1=Alu.mult,
                accum_out=accY[:, t:t + 1])

            if t < T - DVE_TAIL:
                # PE diag: ps += lb_c.T @ [z_c | y_c]
                comb_r = comb[:].rearrange("p (s x) -> p s x", s=2, x=F)
                for c in range(NCt):
                    ps = ps_diag0 if bank[ci] == 0 else ps_diag1
                    nc.tensor.matmul(
                        ps[:, :], lb[:, c * CH:(c + 1) * CH],
                        comb_r[:, :, c * CH:(c + 1) * CH],
                        start=(ci in (0, 1)),
                        stop=(ci in (b0_stop, b1_stop)))
                    ci += 1
            else:
                # DVE-tail: Sx+C and TP partials via fused-accum stt
                tt = t - (T - DVE_TAIL)
                nc.vector.scalar_tensor_tensor(
                    out=junk[:, :Ft], in0=lb[:, :Ft], scalar=0.0,
                    in1=comb[:, :Ft], op0=Alu.bypass, op1=Alu.mult,
                    accum_out=accY[:, T + tt:T + tt + 1])
                nc.vector.scalar_tensor_tensor(
                    out=junk[:, :Ft], in0=lb[:, :Ft], scalar=0.0,
                    in1=comb[:, F:F + Ft], op0=Alu.bypass, op1=Alu.mult,
                    accum_out=accY[:, T + 2 + tt:T + 2 + tt + 1])

        # PSUM -> SBUF (bf16 halves the output DMA) -> DRAM
        nc.scalar.activation(diag_sb[:, :2 * CH], ps_diag0[:, :], Act.Copy)
        nc.vector.tensor_copy(out=diag_sb[:, 2 * CH:], in_=ps_diag1[:, :])

        nc.sync.dma_start(out=out_diag.ap(), in_=diag_sb[:])
        nc.sync.dma_start(out=out_accY.ap(), in_=accY[:])
        nc.sync.dma_start(out=out_accN.ap(), in_=accN[:])

    nc.compile()
    return nc, T


def get_nc():
    if "nc" not in _CACHE:
        _CACHE["nc"] = build_nc()
    return _CACHE["nc"]


def run_device(probs, lbls, trace=False, **run_kwargs):
    """Run the SPMD kernel; returns (per-core result dicts, BassKernelResults)."""
    from concourse import bass_utils

    nc, _ = get_nc()
    probs = np.ascontiguousarray(probs, dtype=np.float32)
    lbls = np.ascontiguousarray(lbls, dtype=np.int32)
    assert probs.shape == (N_TOTAL,) and lbls.shape == (N_TOTAL,)
    M = M_PER_CORE
    in_maps = [
        {"probs": probs[c * M:(c + 1) * M], "lbls": lbls[c * M:(c + 1) * M]}
        for c in range(N_CORES)
    ]
    res = bass_utils.run_bass_kernel_spmd(
        nc, in_maps, core_ids=list(range(N_CORES)), trace=trace, **run_kwargs)
    return res.results, res


def combine(results):
    """Combine per-core partial sums into the final -f1 scalar."""
    Npos = Y = SxC = TP = 0.0
    for r in results:
        dg = np.asarray(r["out_diag"], dtype=np.float64).reshape(P, 2, 2, CH)
        for b in range(2):
            SxC += np.trace(dg[:, b, 0, :])
            TP += np.trace(dg[:, b, 1, :])
        Npos += np.asarray(r["out_accN"], dtype=np.float64).sum()
        v = np.asarray(r["out_accY"], dtype=np.float64)
        T = v.shape[1] - 4
        Y += v[:, :T].sum()
        SxC += v[:, T:T + 2].sum()
        TP += v[:, T + 2:].sum()

    FP = Y - TP
    FN = Npos - SxC + TP
    precision = (TP + EPS) / (TP + FP + EPS)
    recall = (TP + EPS) / (TP + FN + EPS)
    f1 = 2.0 * precision * recall / (precision + recall)
    return np.float32(-f1)


def kernel(probs, lbls):
    results, _ = run_device(probs, lbls)
    return np.asarray(combine(results), dtype=np.float32)


if __name__ == "__main__":
    rng = np.random.default_rng(0)
    p = rng.uniform(0, 1, N_TOTAL).astype(np.float32)
    l = rng.integers(0, 2, N_TOTAL).astype(np.int32)
    out = kernel(p, l)
    print("kernel output:", out)


# revision 25
# speedup vs baseline: 1.1050x; 1.1050x over previous
"""Trainium2 Bass kernel for nn_NegF1: distributed -F1 loss over 16.7M elements.

Data-parallel over 8 NeuronCores; each core streams its 2,097,152-element
slice of probs (f32) / lbls (int32) from HBM (probs on the sync HWDGE ring,
lbls on the scalar ring). Memory-bound, so the structure guarantees the
input stream can never be throttled by compute:

  * Every input tile gets its OWN SBUF slot (no slot recycling; the full
    input fits in SBUF: 2 * 64KB per partition) and ALL input DMA triggers
    are issued up-front, before any compute instruction, so the DMA queues
    are full from the first microsecond and no consumer-side slot-release
    feedback ever gates a DMA.
  * Compute engines just chase the stream; each is far enough under the
    DMA rate to keep up even at its clock-gated slow p-state.

Key algebraic trick: with g = [p > .5], the F1 terms are
  FP = Y - TP,  FN = Npos - (Sx + C) + TP
so Sx = sum(l*p) and C = sum(l*g) are only needed as their SUM, which is
one diag plane z = g + p instead of two.

Work split per [128, F] tile (vs the 5.5us/tile DMA stream rate):
  ACT:  lb = bf16(l), fused accum -> Npos = sum(l)
  DVE:  stt z: (p > .5) + p -> comb[:, :F]          (plane only)
        stt y: (p > .5) * p -> comb[:, F:], accum -> Y
  PE:   diag trick: lhsT = lb chunk, rhs = [z_c | y_c] [128, 2*128] into
        2 alternating PSUM banks; diag block 0 -> Sx + C, block 1 -> TP.
A short burst of zero matmuls warms the PE clock during the first DMAs;
tiles taper at the end to shorten the drain.

Host combine (float64):
  FP = Y - TP;  FN = Npos - SxC + TP
  f1 from TP/FP/FN with eps=1e-5;  return -f1 as float32 scalar.
"""

from contextlib import ExitStack

import numpy as np

N_TOTAL = 16777216
N_CORES = 8
M_PER_CORE = N_TOTAL // N_CORES   # 2097152
P = 128                           # SBUF partitions
EPS = 1e-05
CH = 128                          # diag chunk columns

_CACHE = {}


def build_nc(M=M_PER_CORE, F=2048, comb_bufs=4, warmup_mms=12, debug=False):
    import concourse.bacc as bacc
    import concourse.mybir as mybir
    import concourse.tile as tile

    cols = M // P                 # 16384
    end_taper = [F // 2, F // 4, F // 8, F // 8]
    body = (cols - sum(end_taper)) // F
    tiles = [F] * body + end_taper
    assert sum(tiles) == cols and all(Ft % CH == 0 for Ft in tiles)
    T = len(tiles)

    f32 = mybir.dt.float32
    i32 = mybir.dt.int32
    bf16 = mybir.dt.bfloat16
    Alu = mybir.AluOpType
    Act = mybir.ActivationFunctionType

    nc = bacc.Bacc("TRN2", target_bir_lowering=False, debug=debug,
                   num_devices=N_CORES)

    probs = nc.dram_tensor("probs", [M], f32, kind="ExternalInput")
    lbls = nc.dram_tensor("lbls", [M], i32, kind="ExternalInput")
    out_diag = nc.dram_tensor("out_diag", [P, 2 * 2 * CH], bf16,
                              kind="ExternalOutput")
    out_accN = nc.dram_tensor("out_accN", [P, T], f32, kind="ExternalOutput")
    out_accY = nc.dram_tensor("out_accY", [P, T], f32, kind="ExternalOutput")

    # per-tile DRAM views: tile t is one contiguous block of P*Ft elements
    def tile_view(ap_flat, start_el, Ft):
        return ap_flat[start_el:start_el + P * Ft].rearrange(
            "(p f) -> p f", p=P, f=Ft)

    p1 = probs.ap()
    l1 = lbls.ap()

    with tile.TileContext(nc) as tc, ExitStack() as ctx:
        pin = ctx.enter_context(tc.tile_pool(name="pin", bufs=1))
        lin = ctx.enter_context(tc.tile_pool(name="lin", bufs=1))
        lbpool = ctx.enter_context(tc.tile_pool(name="lbpool", bufs=3))
        cpool = ctx.enter_context(tc.tile_pool(name="cpool", bufs=comb_bufs))
        accp = ctx.enter_context(tc.tile_pool(name="accp", bufs=1))
        psump = ctx.enter_context(tc.tile_pool(name="psump", bufs=1,
                                               space="PSUM"))

        accN = accp.tile([P, T], f32)        # Npos partials (ACT)
        accY = accp.tile([P, T], f32)        # Y partials (DVE)

        # Phase 1: issue EVERY input DMA up-front, all on the SYNC ring,
        # interleaved probs/lbls per tile so the single FIFO delivers tile
        # pairs in order. Each tile has its own statically-assigned slot,
        # so nothing ever waits on a slot release and the stream can never
        # be gated by compute. The HWDGE ring-depth limit only blocks the
        # sync sequencer, which has nothing else to do -- the scalar
        # sequencer runs only ACT ops so they dispatch the moment their
        # data lands.
        pts, lts = [], []
        off = 0
        for t, Ft in enumerate(tiles):
            start_el = P * off
            off += Ft
            pt = pin.tile([P, Ft], f32, tag=f"pt{t}")
            nc.sync.dma_start(out=pt[:, :Ft], in_=tile_view(p1, start_el, Ft))
            lt = lin.tile([P, Ft], i32, tag=f"lt{t}")
            nc.sync.dma_start(out=lt[:, :Ft], in_=tile_view(l1, start_el, Ft))
            pts.append(pt)
            lts.append(lt)

        # two alternating diag accumulators (even / odd chunks) so
        # back-to-back accumulating matmuls don't chain on one PSUM bank
        ps_diag0 = psump.tile([P, 2 * CH], f32)
        ps_diag1 = psump.tile([P, 2 * CH], f32)

        # Warm the PE HAM clock-gate while the first input DMAs stream.
        if warmup_mms:
            wu = accp.tile([P, 2 * CH], bf16)
            nc.vector.memset(wu[:], 0.0)
            ps_wu = psump.tile([P, 2 * CH], f32)
            for i in range(warmup_mms):
                nc.tensor.matmul(ps_wu[:, :], wu[:, :CH], wu[:],
                                 start=(i == 0), stop=(i == warmup_mms - 1))

        nctot = cols // CH              # total diag chunks (128)
        bank = [i % 2 for i in range(nctot)]
        b0_stop = max(i for i, b in enumerate(bank) if b == 0)
        b1_stop = max(i for i, b in enumerate(bank) if b == 1)
        ci = 0
        diag_sb = accp.tile([P, 2 * 2 * CH], bf16)

        # Phase 2: compute, chasing the stream.
        for t, Ft in enumerate(tiles):
            NCt = Ft // CH
            pt, lt = pts[t], lts[t]

            # ACT: lb = bf16(l) with fused accum -> Npos
            lb = lbpool.tile([P, F], bf16, tag="lb")
            nc.scalar.activation(lb[:, :Ft], lt[:, :Ft], Act.Copy,
                                 accum_out=accN[:, t:t + 1])

            # DVE: z = g + p plane; y = g * p plane with fused accum -> Y
            comb = cpool.tile([P, 2 * F], bf16, tag="comb")
            nc.vector.scalar_tensor_tensor(
                out=comb[:, :Ft], in0=pt[:, :Ft], scalar=0.5,
                in1=pt[:, :Ft], op0=Alu.is_gt, op1=Alu.add)
            nc.vector.scalar_tensor_tensor(
                out=comb[:, F:F + Ft], in0=pt[:, :Ft], scalar=0.5,
                in1=pt[:, :Ft], op0=Alu.is_gt, op1=Alu.mult,
                accum_out=accY[:, t:t + 1])

            # PE diag: ps += lb_c.T @ [z_c | y_c]
            comb_r = comb[:].rearrange("p (s x) -> p s x", s=2, x=F)
            for c in range(NCt):
                ps = ps_diag0 if bank[ci] == 0 else ps_diag1
                nc.tensor.matmul(
                    ps[:, :], lb[:, c * CH:(c + 1) * CH],
                    comb_r[:, :, c * CH:(c + 1) * CH],
                    start=(ci in (0, 1)),
                    stop=(ci in (b0_stop, b1_stop)))
                ci += 1

        # PSUM -> SBUF (bf16 halves the output DMA) -> DRAM
        nc.scalar.activation(diag_sb[:, :2 * CH], ps_diag0[:, :], Act.Copy)
        nc.vector.tensor_copy(out=diag_sb[:, 2 * CH:], in_=ps_diag1[:, :])

        nc.sync.dma_start(out=out_diag.ap(), in_=diag_sb[:])
        nc.sync.dma_start(out=out_accY.ap(), in_=accY[:])
        nc.scalar.dma_start(out=out_accN.ap(), in_=accN[:])

    nc.compile()
    return nc, T


def get_nc():
    if "nc" not in _CACHE:
        _CACHE["nc"] = build_nc()
    return _CACHE["nc"]


def run_device(probs, lbls, trace=False, **run_kwargs):
    """Run the SPMD kernel; returns (per-core result dicts, BassKernelResults)."""
    from concourse import bass_utils

    nc, _ = get_nc()
    probs = np.ascontiguousarray(probs, dtype=np.float32)
    lbls = np.ascontiguousarray(lbls, dtype=np.int32)
    assert probs.shape == (N_TOTAL,) and lbls.shape == (N_TOTAL,)
    M = M_PER_CORE
    in_maps = [
        {"probs": probs[c * M:(c + 1) * M], "lbls": lbls[c * M:(c + 1) * M]}
        for c in range(N_CORES)
    ]
    res = bass_utils.run_bass_kernel_spmd(
        nc, in_maps, core_ids=list(range(N_CORES)), trace=trace, **run_kwargs)
    return res.results, res


def combine(results):
    """Combine per-core partial sums into the final -f1 scalar."""
    Npos = Y = SxC = TP = 0.0
    for r in results:
        dg = np.asarray(r["out_diag"], dtype=np.float64).reshape(P, 2, 2, CH)
        for b in range(2):
            SxC += np.trace(dg[:, b, 0, :])
            TP += np.trace(dg[:, b, 1, :])
        Npos += np.asarray(r["out_accN"], dtype=np.float64).sum()
        Y += np.asarray(r["out_accY"], dtype=np.float64).sum()

    FP = Y - TP
    FN = Npos - SxC + TP
    precision = (TP + EPS) / (TP + FP + EPS)
    recall = (TP + EPS) / (TP + FN + EPS)
    f1 = 2.0 * precision * recall / (precision + recall)
    return np.float32(-f1)


def kernel(probs, lbls):
    results, _ = run_device(probs, lbls)
    return np.asarray(combine(results), dtype=np.float32)


if __name__ == "__main__":
    rng = np.random.default_rng(0)
    p = rng.uniform(0, 1, N_TOTAL).astype(np.float32)
    l = rng.integers(0, 2, N_TOTAL).astype(np.int32)
    out = kernel(p, l)
    print("kernel output:", out)
